# revision 2
# baseline (speedup 1.0000x reference)
"""Chamfer loss (chunked) Trainium2 kernel — nn_ChamferLoss_8194797601432.

Reference math: for each 2048-point chunk c of pc1, compute the vector
min over the chunk of ||pc2[m] - pc1_chunk[p]||^2 for all m in pc2 (and
symmetrically for chunks of pc2 vs pc1), concatenate, return
mean(dist1) + mean(dist2)  (scalar fp32).

Device strategy (8 NeuronCores, SPMD, per-core data):
  Core c handles chunk c for both halves (2 tasks per core):
    task := (ref = full opposite cloud [16384 pts], chunk = 2048 pts)
    G[m,p] = -2*ref[m]·chunk[p] + ||chunk[p]||²  via K=21 bf16 matmul rows.
      Precision: each fp32 operand is split into 3 bf16 terms
      (x = b0 + b1 + b2); the 6 dominant cross products per coordinate
      plus a 3-term split of ||chunk||² give |error| ~ 1e-5 absolute,
      which leaves the min-selection bias negligible (fp32r's ~1e-3
      noise biased the min low by ~9e-2 relative — the reason the
      previous version failed).
    Per m-tile (128 ref pts): 4 matmuls (512 chunk pts each) into 4 PSUM
      banks (bufs=8 → 2 m-tiles in flight), ScalarE copies banks 1,3 to
      SBUF, VectorE chains 2 tensor_tensor_scan(min,min) ops
      (in0=PSUM, in1=SBUF copy, 2 elem/lane/cyc), initial-chained.
      Row-groups alternate (0,32) by m-tile parity so consecutive
      m-tiles' matmuls overlap in the PE array.
    Batched extract: 2nd-scan outputs collect in a [128, 8*512] tile;
      one strided VectorE copy pulls all 8 final columns per group.
  Host: add ||ref[m]||², concatenate, mean in float64, cast fp32.
"""

import numpy as np
import ml_dtypes

BF = ml_dtypes.bfloat16

NPTS = 16384
NCHUNK = 2048
NCORES = 8
NM = NPTS // 128  # 128 m-tiles per task
NTASKS = 2
K = 21  # bf16 feature rows
EX = 8  # m-tiles per batched extract

_CACHE = {}


def _build(reps=1):
    import concourse.bacc as bacc
    import concourse.mybir as mybir
    import concourse.tile as tile
    from contextlib import ExitStack

    FP32 = mybir.dt.float32
    BF16 = mybir.dt.bfloat16
    MIN = mybir.AluOpType.min

    nc = bacc.Bacc("TRN2", target_bir_lowering=False)

    refs = [
        nc.dram_tensor(f"ref{t}", [K, NPTS], BF16, kind="ExternalInput")
        for t in range(NTASKS)
    ]
    chunks = [
        nc.dram_tensor(f"chunk{t}", [K, NCHUNK], BF16, kind="ExternalInput")
        for t in range(NTASKS)
    ]
    minout = nc.dram_tensor("minout", [NTASKS, 128, NM], FP32, kind="ExternalOutput")

    with tile.TileContext(nc) as tc:
        with ExitStack() as ctx:
            const_pool = ctx.enter_context(tc.tile_pool(name="const", bufs=1))
            psum_pool = ctx.enter_context(tc.tile_pool(name="psum", bufs=8, space="PSUM"))
            scp_pool = ctx.enter_context(tc.tile_pool(name="scp", bufs=6))
            scra_pool = ctx.enter_context(tc.tile_pool(name="scra", bufs=4))
            scrb_pool = ctx.enter_context(tc.tile_pool(name="scrb", bufs=2))
            out_pool = ctx.enter_context(tc.tile_pool(name="out", bufs=1))

            Rs, Cs, minbufs = [], [], []
            for t in range(NTASKS):
                R = const_pool.tile([128, NPTS], BF16, tag=f"R{t}", name=f"R{t}")
                C = const_pool.tile([128, NCHUNK], BF16, tag=f"C{t}", name=f"C{t}")
                for rg in (0, 32):
                    nc.sync.dma_start(R[rg : rg + K, :], refs[t][:])
                    nc.sync.dma_start(C[rg : rg + K, :], chunks[t][:])
                Rs.append(R)
                Cs.append(C)
                minbufs.append(
                    out_pool.tile([128, NM], FP32, tag=f"mb{t}", name=f"mb{t}")
                )

            loop_cm = tc.For_i(0, reps, 1) if reps > 1 else None
            if loop_cm is not None:
                loop_cm.__enter__()

            for t in range(NTASKS):
                R, C, minbuf = Rs[t], Cs[t], minbufs[t]
                scrb = None
                for mt in range(NM):
                    rg = 32 * (mt % 2)
                    slot = mt % EX
                    if slot == 0:
                        scrb = scrb_pool.tile(
                            [128, EX * 512], FP32, tag="scrb", name=f"scrb_{t}_{mt}"
                        )
                    lhsT = R[rg : rg + K, mt * 128 : (mt + 1) * 128]
                    ps = []
                    for p in range(4):
                        pst = psum_pool.tile(
                            [128, 512], FP32, tag="ps", name=f"ps_{t}_{mt}_{p}"
                        )
                        nc.tensor.matmul(
                            pst[:],
                            lhsT=lhsT,
                            rhs=C[rg : rg + K, p * 512 : (p + 1) * 512],
                            start=True,
                            stop=True,
                            tile_position=(rg, 0),
                        )
                        ps.append(pst)
                    scp1 = scp_pool.tile([128, 512], FP32, tag="scp", name=f"scp1_{t}_{mt}")
                    nc.scalar.copy(scp1[:], ps[1][:])
                    scra = scra_pool.tile([128, 512], FP32, tag="scra", name=f"scra_{t}_{mt}")
                    nc.vector.tensor_tensor_scan(
                        scra[:], ps[0][:], scp1[:], initial=1e30, op0=MIN, op1=MIN
                    )
                    scp3 = scp_pool.tile([128, 512], FP32, tag="scp", name=f"scp3_{t}_{mt}")
                    nc.scalar.copy(scp3[:], ps[3][:])
                    nc.vector.tensor_tensor_scan(
                        scrb[:, slot * 512 : (slot + 1) * 512],
                        ps[2][:],
                        scp3[:],
                        initial=scra[:, 511:512],
                        op0=MIN,
                        op1=MIN,
                    )
                    if slot == EX - 1:
                        nc.vector.tensor_copy(
                            minbuf[:, mt - EX + 1 : mt + 1], scrb[:, 511::512]
                        )

            if loop_cm is not None:
                loop_cm.__exit__(None, None, None)

            for t in range(NTASKS):
                nc.sync.dma_start(minout[t], minbufs[t][:])

    nc.compile()
    return nc


def get_nc(reps=1):
    if reps not in _CACHE:
        _CACHE[reps] = _build(reps)
    return _CACHE[reps]


def _split3(x):
    """fp32 array -> three bf16 arrays with b0+b1+b2 ~ x (residual ~2^-27|x|)."""
    x = x.astype(np.float32)
    b0 = x.astype(BF)
    r1 = x - b0.astype(np.float32)
    b1 = r1.astype(BF)
    r2 = r1 - b1.astype(np.float32)
    b2 = r2.astype(BF)
    return b0, b1, b2


def _ref_feat(p):
    """p [N,3] fp32 -> [K, N] bf16 ref-side rows."""
    r0, r1, r2 = _split3(p.T)  # each [3, N]
    one = np.ones((p.shape[0],), BF)
    rows = []
    for d in range(3):
        rows += [r0[d], r0[d], r1[d], r0[d], r2[d], r1[d]]
    rows += [one, one, one]
    return np.stack(rows)


def _chunk_feat(p):
    """p [n,3] fp32 -> [K, n] bf16 chunk-side rows."""
    s0, s1, s2 = _split3(-2.0 * p.T)  # each [3, n]
    v = (p.astype(np.float64) ** 2).sum(-1).astype(np.float32)
    v0, v1, v2 = _split3(v)
    rows = []
    for d in range(3):
        rows += [s0[d], s1[d], s0[d], s2[d], s0[d], s1[d]]
    rows += [v0, v1, v2]
    return np.stack(rows)


def _prep_in_maps(pc1, pc2):
    refA = _ref_feat(pc2)  # dist1: ref = pc2, chunks of pc1
    refB = _ref_feat(pc1)  # dist2: ref = pc1, chunks of pc2
    in_maps = []
    for c in range(NCORES):
        in_maps.append(
            {
                "ref0": refA,
                "chunk0": _chunk_feat(pc1[c * NCHUNK : (c + 1) * NCHUNK]),
                "ref1": refB,
                "chunk1": _chunk_feat(pc2[c * NCHUNK : (c + 1) * NCHUNK]),
            }
        )
    return in_maps


def run_on_device(in_maps, reps=1):
    from concourse.bass_utils import run_bass_kernel_spmd

    nc = get_nc(reps)
    res = run_bass_kernel_spmd(nc, in_maps, core_ids=list(range(NCORES)))
    return res.results


def _postprocess(results, pc1, pc2):
    n2_1 = (pc1.astype(np.float64) ** 2).sum(-1)
    n2_2 = (pc2.astype(np.float64) ** 2).sum(-1)
    d1 = np.empty((NCORES, NPTS), np.float64)
    d2 = np.empty((NCORES, NPTS), np.float64)
    for c in range(NCORES):
        mo = results[c]["minout"].astype(np.float64)  # [2, 128, NM]
        d1[c] = mo[0].T.reshape(-1) + n2_2
        d2[c] = mo[1].T.reshape(-1) + n2_1
    return np.array(d1.mean() + d2.mean(), dtype=np.float32)


def kernel(output_pc, gt_pc):
    pc1 = np.asarray(output_pc, dtype=np.float32).reshape(NPTS, 3)
    pc2 = np.asarray(gt_pc, dtype=np.float32).reshape(NPTS, 3)
    in_maps = _prep_in_maps(pc1, pc2)
    results = run_on_device(in_maps)
    return _postprocess(results, pc1, pc2)
